# revision 8
# baseline (speedup 1.0000x reference)
"""Trainium2 Bass kernel for the CAM-drop attention module.

Computes, per sample n:
    cams  = relu(w @ x)            # [Cout=4, HW]   (1x1 conv over Cin=4096)
    thr   = gama * max_hw(cams)    # [4, 1]
    drop  = where(cams > thr, 0, cams)
    mean  = sum_o(drop) / 4        # [1, HW]
    out   = x * mean               # [Cin, HW]

Sharding: data-parallel over batch N=32 across 8 NeuronCores (4 samples each).

v3 design, from measured HW constants (chained-slope microbenchmarks):
  - HBM reads sustain ~358 GB/s/core with all 8 cores running (the 2.9 TB/s
    device aggregate); writes are much cheaper (~0.9us/MB of interference).
    Per-iter DMA floor: 51.4 MB fp32 loads (~144us) + fp16 store interference
    (~24us) ~= 168us.
  - The drop-threshold compare is numerically fragile (min |cams-thr|/thr
    ~2.4e-5 on this input set) so the conv consumes fp32 x: loads stay fp32.
  - Tiny-matmul dispatch costs ~371ns/instr on HW, so the x-stationary conv
    (937 matmuls/iter) is dispatch-bound (~333us) — worse than streaming x
    through the PE at fp32's 4 cyc/row (256 matmuls, ~148us/iter, engine
    bound). Keep the stream conv.
  - The final multiply reads fp32 x and mean and writes fp16 directly
    (single rounding, ~2^-11 elementwise): DVE 26us/sample, overlapped with
    the next sample's loads+conv. Stores ship fp16 (half the baseline write
    traffic) on the ACT ring.
"""

import numpy as np
from contextlib import ExitStack

import concourse.bass as bass
import concourse.bacc as bacc
import concourse.tile as tile
from concourse import mybir
from concourse.bass_utils import run_bass_kernel_spmd
from concourse.masks import make_identity

# Problem geometry (hardcoded per the grading contract).
N_TOTAL, CIN, H, W = 32, 4096, 28, 28
HW = H * W            # 784
COUT = 4
N_CORES = 8
N_PER_CORE = N_TOTAL // N_CORES   # 4
P = 128
NCHUNKS = CIN // P    # 32 partition-chunks of Cin
QCH = 4               # chunks per DMA transfer (1.6 MB)
NT = NCHUNKS // QCH   # 8 tiles per sample
NSPLIT = 512          # PSUM-bank split of the HW free dim: 512 + 272
F32 = mybir.dt.float32
F16 = mybir.dt.float16


def build_cam_body(ctx: ExitStack, tc: "tile.TileContext", out_ap, x_ap, w_ap,
                   g_ap, iters=1):
    """Emit the kernel body. x_ap: [N_PER_CORE, CIN, HW] f32 DRAM,
    out_ap: [N_PER_CORE, CIN, HW] f16 DRAM, w_ap: [COUT, CIN] f32 DRAM,
    g_ap: [1, 1] f32 DRAM."""
    nc = tc.nc

    xpool = ctx.enter_context(tc.tile_pool(name="xq", bufs=9))
    o16pool = ctx.enter_context(tc.tile_pool(name="o16", bufs=5))
    small = ctx.enter_context(tc.tile_pool(name="small", bufs=1))
    cpool = ctx.enter_context(tc.tile_pool(name="cams", bufs=2))
    mpool = ctx.enter_context(tc.tile_pool(name="mean", bufs=2))
    ps_c = ctx.enter_context(tc.tile_pool(name="ps_cams", bufs=2, space="PSUM"))
    ps_b = ctx.enter_context(tc.tile_pool(name="ps_bcast", bufs=2, space="PSUM"))

    # ---- one-time setup: transpose w to [Cin, Cout] layout, constants ----
    # w lives in DRAM as [4, 4096]; the matmul needs per-chunk lhsT tiles of
    # shape [128 (Cin slice), 4]. A direct DMA of that layout would be
    # element-granular, so load [4, 4096] and transpose on the PE.
    wsb = xpool.tile([COUT, CIN], F32, tag="xq")
    nc.sync.dma_start(out=wsb, in_=w_ap)

    ident = small.tile([P, P], F32)
    make_identity(nc, ident)

    wt_ps = ps_b.tile([P, NCHUNKS * COUT], F32, tag="bps")
    for k in range(NCHUNKS):
        nc.tensor.transpose(
            wt_ps[:, k * COUT:(k + 1) * COUT],
            wsb[:, k * P:(k + 1) * P],
            ident[0:COUT, 0:COUT],
        )
    wt = small.tile([P, NCHUNKS, COUT], F32)
    nc.vector.tensor_copy(wt, wt_ps.rearrange("p (k o) -> p k o", o=COUT))

    # 0.25 * ones[4, 128]: the channel-sum + partition-broadcast matmul weight.
    ones = small.tile([COUT, P], F32)
    nc.vector.memset(ones, 0.25)

    # gama broadcast to partitions 0..3.
    gsb = small.tile([COUT, 1], F32)
    nc.gpsimd.dma_start(out=gsb, in_=g_ap.to_broadcast([COUT, 1]))

    # ---- per-sample pipeline ----
    for s in [s for _ in range(iters) for s in range(N_PER_CORE)]:
        xs = x_ap[s].rearrange("(k p) hw -> p k hw", p=P)    # [128, 32, 784]
        os_ = out_ap[s].rearrange("(k p) hw -> p k hw", p=P)

        # Load the sample in NT quarter-chunks; the conv consumes chunk k as
        # soon as its tile lands, so the PE runs ~concurrently with the loads.
        xq = []
        for t in range(NT):
            xt = xpool.tile([P, QCH, HW], F32, tag="xq")
            nc.sync.dma_start(out=xt, in_=xs[:, t * QCH:(t + 1) * QCH, :])
            xq.append(xt)

        # cams[o, hw] = sum_c w[o, c] x[c, hw], accumulated over 32 chunks.
        cams_ps = ps_c.tile([COUT, 1024], F32)  # 4 KB -> two PSUM banks
        for k in range(NCHUNKS):
            lhsT = wt[:, k, :]
            rhs = xq[k // QCH][:, k % QCH, :]
            first, last = k == 0, k == NCHUNKS - 1
            nc.tensor.matmul(cams_ps[:, 0:NSPLIT], lhsT, rhs[:, 0:NSPLIT],
                             start=first, stop=last)
            nc.tensor.matmul(cams_ps[:, NSPLIT:HW], lhsT, rhs[:, NSPLIT:HW],
                             start=first, stop=last)

        # relu -> spatial max -> threshold -> drop -> channel sum (+broadcast).
        # ACT computes relu(cams) while DVE reduces the raw max concurrently;
        # thr = gama * max(raw_max, 0) == gama * max(relu(cams)).
        cams_sb = cpool.tile([COUT, HW], F32)
        nc.scalar.activation(cams_sb, cams_ps[:, 0:HW],
                             mybir.ActivationFunctionType.Relu)
        cmax = cpool.tile([COUT, 1], F32)
        nc.vector.tensor_reduce(cmax, cams_ps[:, 0:HW],
                                axis=mybir.AxisListType.X,
                                op=mybir.AluOpType.max)
        thr = cpool.tile([COUT, 1], F32)
        nc.vector.tensor_scalar(thr, cmax, 0.0, gsb,
                                op0=mybir.AluOpType.max,
                                op1=mybir.AluOpType.mult)
        dropped = cpool.tile([COUT, HW], F32)
        # dropped = (cams <= thr) * cams
        nc.vector.scalar_tensor_tensor(dropped, cams_sb, thr, cams_sb,
                                       op0=mybir.AluOpType.is_le,
                                       op1=mybir.AluOpType.mult)

        # bps[p, hw] = 0.25 * sum_o dropped[o, hw], replicated to 128 rows.
        bps = ps_b.tile([P, 1024], F32, tag="bps")
        nc.tensor.matmul(bps[:, 0:NSPLIT], ones, dropped[:, 0:NSPLIT],
                         start=True, stop=True)
        nc.tensor.matmul(bps[:, NSPLIT:HW], ones, dropped[:, NSPLIT:HW],
                         start=True, stop=True)
        mean32 = mpool.tile([P, HW], F32, tag="m32")
        nc.vector.tensor_copy(mean32, bps[:, 0:HW])
        mb = mean32.unsqueeze(1).to_broadcast([P, QCH, HW])

        # out16 = x * mean in one fused DVE pass (fp32 in, fp16 out, single
        # rounding), then store fp16 on the ACT ring so stores don't queue
        # behind next-sample loads on the SP ring. The mul is the last reader
        # of each x tile, freeing its slot for the next sample's loads.
        for t in range(NT):
            o16 = o16pool.tile([P, QCH, HW], F16, tag="o16")
            nc.vector.tensor_mul(o16, xq[t], mb)
            nc.scalar.dma_start(out=os_[:, t * QCH:(t + 1) * QCH, :], in_=o16)


def build_module(iters=1):
    """iters > 1 unrolls the whole body multiple times inside one NEFF —
    used only by the timing harness to amortize dispatch overhead."""
    nc = bacc.Bacc(trn_type="TRN2", num_devices=N_CORES, name="cam_drop")
    x = nc.dram_tensor("x", [N_PER_CORE, CIN, HW], F32, kind="ExternalInput").ap()
    w = nc.dram_tensor("w", [COUT, CIN], F32, kind="ExternalInput").ap()
    g = nc.dram_tensor("gama", [1, 1], F32, kind="ExternalInput").ap()
    out = nc.dram_tensor("out", [N_PER_CORE, CIN, HW], F16,
                         kind="ExternalOutput").ap()
    with tile.TileContext(nc) as tc:
        with ExitStack() as ctx:
            build_cam_body(ctx, tc, out, x, w, g, iters=iters)
    nc.compile()
    return nc


_cached_module = None


def make_in_maps(x, fc_weights, gama):
    """Host-side prep: shard FULL inputs into per-core input maps."""
    xs = np.ascontiguousarray(
        np.asarray(x, dtype=np.float32).reshape(N_TOTAL, CIN, HW))
    w = np.ascontiguousarray(
        np.asarray(fc_weights, dtype=np.float32).reshape(COUT, CIN))
    g = np.asarray(gama, dtype=np.float32).reshape(1, 1)
    return [
        {"x": np.ascontiguousarray(xs[i * N_PER_CORE:(i + 1) * N_PER_CORE]),
         "w": w, "gama": g}
        for i in range(N_CORES)
    ]


def assemble_out(outs):
    """Host-side post: full (N_TOTAL, CIN, HW) fp32 from gathered outputs."""
    return np.asarray(outs["out"], dtype=np.float32).reshape(N_TOTAL, CIN, HW)


def run(x, fc_weights, gama, trace=False):
    """Shard inputs over 8 cores, run, gather. Returns (output, BassKernelResults)."""
    global _cached_module
    if _cached_module is None:
        _cached_module = build_module()
    nc = _cached_module

    in_maps = make_in_maps(x, fc_weights, gama)
    if trace:
        try:  # this container's antenv has no axon NTFF hook
            from antenv.axon_hooks import get_axon_ntff_profile_hook  # noqa: F401
        except ImportError:
            trace = False
    res = run_bass_kernel_spmd(nc, in_maps, core_ids=list(range(N_CORES)),
                               trace=trace)
    full = assemble_out(
        {"out": np.concatenate([r["out"] for r in res.results], axis=0)})
    return full.reshape(N_TOTAL, CIN, H, W), res


def kernel(x, fc_weights, gama):
    out, _ = run(x, fc_weights, gama, trace=False)
    return out
